# revision 4
# baseline (speedup 1.0000x reference)
"""BilinearAttention (B=2, T=2048, C=1024, H=16, D=64) on 8 TRN2 NeuronCores.

Sharding: batch*heads across the 8 cores — core c handles batch b = c//4 and
heads [4*(c%4), 4*(c%4)+4).  Each core computes its heads' attention and a
partial output projection; the host sums the four per-batch partials.

Per-core kernel (Bass/Tile, all matmuls float32r = full PE rate, tf32-ish):
  phase P: xT chunk-streamed; q1/k1/q2/k2 projections in [o, t] layout; RoPE
           applied as q*cos + swap(q*sin') where the half-swap runs on the PE
           (permutation-matmul accumulated with an identity-matmul);
           v projected in [t, d] layout.  1/D^2 pattern scale is folded into
           Wv (exact power-of-two scaling).
  phase A: per head, scores computed transposed (patT[tk, tq]); the s1/s2
           matmuls of one head are row-packed on the PE (two concurrent K=64
           matmuls at row groups 0/64 — q2/k2 weights are head-swapped on the
           host to make the row groups line up).  ScalarE copies s1 from PSUM,
           VectorE forms s1*s2 with the causal mask (precomputed 128x128
           triangle applied only on diagonal tiles); the V-matmul accumulates
           yT[d, tq] in PSUM over tk tiles (strictly-upper tiles skipped).
  phase O: out[t, :] = yT.T @ WoutT slice, interleaved per chunk with phase A.
"""
import time
import numpy as np
import ml_dtypes

import concourse.bass as bass
import concourse.mybir as mybir
from concourse.tile import TileContext
from concourse.bass_utils import run_bass_kernel_spmd

F32 = mybir.dt.float32
F32R = mybir.dt.float32r

N_HEAD = 16
N_EMBD = 1024
HEAD_DIM = 64
ROPE_BASE = 10000.0
T_SEQ = 2048


def _legalize_waits(nc, max_waits=1):
    """walrus (CoreV3) accepts at most one sync-wait per instruction; hoist
    extras onto same-engine NoOps immediately before the instruction."""
    n_split = 0
    for f in nc.m.functions:
        for bb in f.blocks:
            insts = bb.instructions
            new_insts = []
            changed = False
            for inst in insts:
                si = inst.sync_info
                waits = list(si.on_wait) if si and si.on_wait else []
                if len(waits) > max_waits:
                    extra = waits[:-max_waits]
                    keep = waits[-max_waits:]
                    for i in range(0, len(extra), max_waits):
                        nop = mybir.InstNoOp(
                            name=f"{inst.name}_ws{i}",
                            engine=inst.engine,
                            sync_info=mybir.SyncInfo(
                                on_wait=extra[i:i + max_waits], on_update=[]
                            ),
                            bass_nofuse=True,
                        )
                        new_insts.append(nop)
                    inst.sync_info = mybir.SyncInfo(
                        on_wait=keep, on_update=list(si.on_update or [])
                    )
                    changed = True
                    n_split += 1
                new_insts.append(inst)
            if changed:
                bb.instructions = new_insts
    return n_split


def _build_program(T=T_SEQ):
    CH = 512
    NJ = T // CH
    NT = T // 128
    nc = bass.Bass("TRN2")

    xT = nc.dram_tensor("xT", [8, 128, T], F32R, kind="ExternalInput").ap()
    wqk = nc.dram_tensor("wqk", [8, 128, 1024], F32R, kind="ExternalInput").ap()
    wv = nc.dram_tensor("wv", [8, 128, 256], F32R, kind="ExternalInput").ap()
    wo = nc.dram_tensor("wo", [2, 128, 1024], F32R, kind="ExternalInput").ap()
    cc = nc.dram_tensor("cc", [128, T], F32, kind="ExternalInput").ap()
    ss = nc.dram_tensor("ss", [128, T], F32, kind="ExternalInput").ap()
    sid = nc.dram_tensor("sid", [128, 128], F32R, kind="ExternalInput").ap()
    iden = nc.dram_tensor("iden", [128, 128], F32R, kind="ExternalInput").ap()
    tri = nc.dram_tensor("tri", [128, 128], F32, kind="ExternalInput").ap()
    out = nc.dram_tensor("out", [T, 1024], F32, kind="ExternalOutput").ap()

    with TileContext(nc) as tc:
        with tc.tile_pool(name="persist", bufs=1) as persist:
            qkrot = [[persist.tile([128, T], F32R, name=f"qkrot_{s}_{g}")
                      for g in range(2)] for s in range(4)]
            v_sb = persist.tile([128, NT, 256], F32R, name="v_sb")
            tri_sb = persist.tile([128, 128], F32, name="tri_sb")
            sid_sb = persist.tile([128, 128], F32R, name="sid_sb")
            iden_sb = persist.tile([128, 128], F32R, name="iden_sb")
            nc.sync.dma_start(out=tri_sb[:], in_=tri)
            nc.sync.dma_start(out=sid_sb[:], in_=sid)
            nc.sync.dma_start(out=iden_sb[:], in_=iden)

            # ---------------- phase P: projections + rope ----------------
            with tc.tile_pool(name="projw", bufs=1) as projw, \
                 tc.tile_pool(name="xcp", bufs=2) as xcp, \
                 tc.tile_pool(name="ropet", bufs=4) as ropet, \
                 tc.tile_pool(name="psq", bufs=3, space="PSUM") as psq, \
                 tc.tile_pool(name="psr", bufs=2, space="PSUM") as psr, \
                 tc.tile_pool(name="psv", bufs=2, space="PSUM") as psv:
                wqk_sb = projw.tile([128, 8, 1024], F32R, name="wqk_sb")
                wv_sb = projw.tile([128, 8, 256], F32R, name="wv_sb")
                cc_sb = projw.tile([128, T], F32, name="cc_sb")
                ss_sb = projw.tile([128, T], F32, name="ss_sb")
                for k in range(8):
                    nc.sync.dma_start(out=wqk_sb[:, k, :], in_=wqk[k])
                    nc.sync.dma_start(out=wv_sb[:, k, :], in_=wv[k])
                nc.sync.dma_start(out=cc_sb[:], in_=cc)
                nc.sync.dma_start(out=ss_sb[:], in_=ss)

                for j in range(NJ):
                    jsl = bass.ts(j, CH)
                    xc = xcp.tile([128, 8, CH], F32R, name="xc")
                    for k in range(8):
                        nc.sync.dma_start(out=xc[:, k, :], in_=xT[k, :, jsl])
                    for tt in range(4):
                        pv = psv.tile([128, 256], F32, name="pv")
                        for k in range(8):
                            nc.tensor.matmul(
                                pv[:], xc[:, k, bass.ts(tt, 128)], wv_sb[:, k, :],
                                start=(k == 0), stop=(k == 7))
                        nc.scalar.copy(out=v_sb[:, j * 4 + tt, :], in_=pv[:])
                    for s in range(4):
                        for g in range(2):
                            col = s * 256 + g * 128
                            pq = psq.tile([128, CH], F32, name="pq")
                            for k in range(8):
                                nc.tensor.matmul(
                                    pq[:], wqk_sb[:, k, col:col + 128], xc[:, k, :],
                                    start=(k == 0), stop=(k == 7))
                            wt = ropet.tile([128, CH], F32R, tag="wt", name="wt")
                            ut = ropet.tile([128, CH], F32R, tag="ut", name="ut")
                            nc.vector.tensor_mul(wt[:], pq[:], ss_sb[:, jsl])
                            nc.vector.tensor_mul(ut[:], pq[:], cc_sb[:, jsl])
                            pr = psr.tile([128, CH], F32, name="pr")
                            nc.tensor.matmul(pr[:], sid_sb[:], wt[:],
                                             start=True, stop=False)
                            nc.tensor.matmul(pr[:], iden_sb[:], ut[:],
                                             start=False, stop=True)
                            nc.scalar.copy(out=qkrot[s][g][:, jsl], in_=pr[:])

            # ---------------- phase A + O ----------------
            with tc.tile_pool(name="patp", bufs=4) as patp, \
                 tc.tile_pool(name="s1cp", bufs=4) as s1cp, \
                 tc.tile_pool(name="tmpp", bufs=2) as tmpp, \
                 tc.tile_pool(name="outp", bufs=3) as outp, \
                 tc.tile_pool(name="attnp", bufs=1) as attnp, \
                 tc.tile_pool(name="pss", bufs=5, space="PSUM") as pss, \
                 tc.tile_pool(name="psy", bufs=2, space="PSUM") as psy, \
                 tc.tile_pool(name="pso", bufs=1, space="PSUM") as pso:
                yT = [attnp.tile([128, T], F32R, name=f"yT_{p}") for p in range(2)]
                wo_sb = attnp.tile([128, 2, 1024], F32R, name="wo_sb")
                nc.sync.dma_start(out=wo_sb[:], in_=wo.rearrange("p q n -> q p n"))
                for j in range(NJ):
                    jsl = bass.ts(j, CH)
                    for pair in range(2):
                        ytiles = [psy.tile([64, CH], F32, tag="y", name=f"yp{h}")
                                  for h in range(2)]
                        ni = 4 * j + 4
                        for i in range(ni):
                            diag = i >= 4 * j
                            c0 = 128 * (i - 4 * j) if diag else 0
                            isl = bass.ts(i, 128)
                            for h in range(2):
                                hrow = bass.ts(h, 64)
                                hrow2 = bass.ts(1 - h, 64)
                                s1 = pss.tile([128, CH], F32, tag="s", name="s1")
                                s2 = pss.tile([128, CH], F32, tag="s", name="s2")
                                nc.tensor.matmul(
                                    s1[:], qkrot[1][pair][hrow, isl],
                                    qkrot[0][pair][hrow, jsl],
                                    start=True, stop=True)
                                nc.tensor.matmul(
                                    s2[:], qkrot[3][pair][hrow2, isl],
                                    qkrot[2][pair][hrow2, jsl],
                                    start=True, stop=True)
                                s1c = s1cp.tile([128, CH], F32, name="s1c")
                                pat = patp.tile([128, CH], F32R, name="pat")
                                if not diag:
                                    nc.scalar.copy(out=s1c[:], in_=s1[:])
                                    nc.vector.tensor_mul(pat[:], s1c[:], s2[:])
                                else:
                                    nc.scalar.copy(out=s1c[:, c0:], in_=s1[:, c0:])
                                    tmp = tmpp.tile([128, 128], F32, name="tmp")
                                    nc.vector.tensor_mul(
                                        tmp[:], s2[:, c0:c0 + 128], tri_sb[:])
                                    nc.vector.tensor_mul(
                                        pat[:, c0:c0 + 128],
                                        s1c[:, c0:c0 + 128], tmp[:])
                                    if c0 + 128 < CH:
                                        nc.vector.tensor_mul(
                                            pat[:, c0 + 128:],
                                            s1c[:, c0 + 128:], s2[:, c0 + 128:])
                                nc.tensor.matmul(
                                    ytiles[h][:, c0:],
                                    v_sb[:, i, bass.ts(2 * pair + h, 64)],
                                    pat[:, c0:],
                                    start=(i == 0), stop=(i == ni - 1),
                                    skip_group_check=True)
                        for h in range(2):
                            nc.scalar.copy(
                                out=yT[pair][bass.ts(h, 64), jsl],
                                in_=ytiles[h][:])
                    for tt in range(4 * j, 4 * j + 4):
                        tsl = bass.ts(tt, 128)
                        for co in range(2):
                            po = pso.tile([128, 512], F32, name="po")
                            nc.tensor.matmul(po[:], yT[0][:, tsl],
                                             wo_sb[:, 0, bass.ts(co, 512)],
                                             start=True, stop=False)
                            nc.tensor.matmul(po[:], yT[1][:, tsl],
                                             wo_sb[:, 1, bass.ts(co, 512)],
                                             start=False, stop=True)
                            ot = outp.tile([128, 512], F32, name="ot")
                            nc.scalar.copy(out=ot[:], in_=po[:])
                            nc.sync.dma_start(
                                out=out[tsl, bass.ts(co, 512)], in_=ot[:])
    return nc


# ------------------------------------------------------------- host side ---
def _rope_tables(T):
    inv_freq = (1.0 / (ROPE_BASE ** (np.arange(0, HEAD_DIM, 2, dtype=np.float32)
                                     / np.float32(HEAD_DIM)))).astype(np.float32)
    t = np.arange(T, dtype=np.float32)
    freqs = (t[:, None] * inv_freq[None, :]).astype(np.float32)
    cos = np.cos(freqs).astype(ml_dtypes.bfloat16).astype(np.float32)
    sin = np.sin(freqs).astype(ml_dtypes.bfloat16).astype(np.float32)
    cosT, sinT = cos.T, sin.T
    cc = np.ascontiguousarray(np.concatenate([cosT, cosT, cosT, cosT], axis=0))
    ss = np.ascontiguousarray(np.concatenate([-sinT, sinT, -sinT, sinT], axis=0))
    return cc, ss


def _const_tables():
    sid = np.zeros((128, 128), dtype=np.float32)
    for blk in range(2):
        for m in range(32):
            sid[blk * 64 + m + 32, blk * 64 + m] = 1.0
            sid[blk * 64 + m, blk * 64 + m + 32] = 1.0
    iden = np.eye(128, dtype=np.float32)
    r = np.arange(128)
    tri = (r[None, :] >= r[:, None]).astype(np.float32)
    return sid, iden, tri


def _make_in_maps(x, Wq1, Wk1, Wq2, Wk2, Wv, Wout, T):
    cc, ss = _rope_tables(T)
    sid, iden, tri = _const_tables()
    in_maps = []
    for core in range(8):
        b = core // 4
        hs = (core % 4) * 4
        xTb = np.ascontiguousarray(x[b].T).reshape(8, 128, T)
        cols = []
        for s, W in enumerate((Wq1, Wk1, Wq2, Wk2)):
            for g in range(2):
                hA, hB = hs + 2 * g, hs + 2 * g + 1
                if s >= 2:
                    hA, hB = hB, hA
                cols.append(W[hA * 64:(hA + 1) * 64, :].T)
                cols.append(W[hB * 64:(hB + 1) * 64, :].T)
        wqk = np.ascontiguousarray(
            np.concatenate(cols, axis=1)).reshape(8, 128, 1024)
        wv = np.ascontiguousarray(
            Wv[hs * 64:(hs + 4) * 64, :].T * np.float32(2.0 ** -12)
        ).reshape(8, 128, 256)
        wo = np.ascontiguousarray(
            Wout[:, hs * 64:(hs + 4) * 64].T).reshape(2, 128, 1024)
        in_maps.append({
            "xT": xTb.astype(np.float32), "wqk": wqk.astype(np.float32),
            "wv": wv.astype(np.float32), "wo": wo.astype(np.float32),
            "cc": cc, "ss": ss, "sid": sid, "iden": iden, "tri": tri,
        })
    return in_maps


_CACHED_NC = None


def kernel(x, Wq1, Wk1, Wq2, Wk2, Wv, Wout):
    global _CACHED_NC
    x = np.asarray(x, dtype=np.float32)
    args = [np.asarray(a, dtype=np.float32) for a in
            (Wq1, Wk1, Wq2, Wk2, Wv, Wout)]
    T = x.shape[1]
    if _CACHED_NC is None:
        nc = _build_program(T)
        _legalize_waits(nc, max_waits=1)
        _CACHED_NC = nc
    in_maps = _make_in_maps(x, *args, T)
    res = None
    last_err = None
    for attempt in range(3):
        try:
            res = run_bass_kernel_spmd(_CACHED_NC, in_maps, list(range(8)))
            break
        except Exception as e:  # transient NRT exec-unit wedge: retry
            last_err = e
            time.sleep(2.0)
    if res is None:
        raise last_err
    out = np.zeros((2, T, 1024), dtype=np.float32)
    for core in range(8):
        out[core // 4] += res.results[core]["out"]
    return out


# revision 5
# speedup vs baseline: 1.3343x; 1.3343x over previous
"""BilinearAttention (B=2, T=2048, C=1024, H=16, D=64) on 8 TRN2 NeuronCores.

Sharding: batch*heads across the 8 cores — core c handles batch b = c//4 and
heads [4*(c%4), 4*(c%4)+4).  Each core computes its heads' attention and a
partial output projection; the host sums the four per-batch partials.

Per-core kernel (Bass/Tile, all matmuls float32r = full PE rate, tf32-ish):
  phase P: xT chunk-streamed; q1/k1/q2/k2 projections in [o, t] layout; RoPE
           applied as q*cos + swap(q*sin') where the half-swap runs on the PE
           (permutation-matmul accumulated with an identity-matmul);
           v projected in [t, d] layout.  1/D^2 pattern scale is folded into
           Wv (exact power-of-two scaling).
  phase A: per head, scores computed transposed (patT[tk, tq]); the s1/s2
           matmuls of one head are row-packed on the PE (two concurrent K=64
           matmuls at row groups 0/64 — q2/k2 weights are head-swapped on the
           host to make the row groups line up).  ScalarE copies s1 from PSUM,
           VectorE forms s1*s2 with the causal mask (precomputed 128x128
           triangle applied only on diagonal tiles); the V-matmul accumulates
           yT[d, tq] in PSUM over tk tiles (strictly-upper tiles skipped).
  phase O: out[t, :] = yT.T @ WoutT slice, interleaved per chunk with phase A.
"""
import time
import numpy as np
import ml_dtypes

import concourse.bass as bass
import concourse.mybir as mybir
from concourse.tile import TileContext
from concourse.bass_utils import run_bass_kernel_spmd

F32 = mybir.dt.float32
F32R = mybir.dt.float32r

N_HEAD = 16
N_EMBD = 1024
HEAD_DIM = 64
ROPE_BASE = 10000.0
T_SEQ = 2048


def _legalize_waits(nc, max_waits=1):
    """walrus (CoreV3) accepts at most one sync-wait per instruction; hoist
    extras onto same-engine NoOps immediately before the instruction."""
    n_split = 0
    for f in nc.m.functions:
        for bb in f.blocks:
            insts = bb.instructions
            new_insts = []
            changed = False
            for inst in insts:
                si = inst.sync_info
                waits = list(si.on_wait) if si and si.on_wait else []
                if len(waits) > max_waits:
                    extra = waits[:-max_waits]
                    keep = waits[-max_waits:]
                    for i in range(0, len(extra), max_waits):
                        nop = mybir.InstNoOp(
                            name=f"{inst.name}_ws{i}",
                            engine=inst.engine,
                            sync_info=mybir.SyncInfo(
                                on_wait=extra[i:i + max_waits], on_update=[]
                            ),
                            bass_nofuse=True,
                        )
                        new_insts.append(nop)
                    inst.sync_info = mybir.SyncInfo(
                        on_wait=keep, on_update=list(si.on_update or [])
                    )
                    changed = True
                    n_split += 1
                new_insts.append(inst)
            if changed:
                bb.instructions = new_insts
    return n_split


def _build_program(T=T_SEQ):
    CH = 512
    NJ = T // CH
    NT = T // 128
    nc = bass.Bass("TRN2")

    xT = nc.dram_tensor("xT", [8, 128, T], F32R, kind="ExternalInput").ap()
    wqk = nc.dram_tensor("wqk", [8, 128, 1024], F32R, kind="ExternalInput").ap()
    wv = nc.dram_tensor("wv", [8, 128, 256], F32R, kind="ExternalInput").ap()
    wo = nc.dram_tensor("wo", [2, 128, 1024], F32R, kind="ExternalInput").ap()
    cc = nc.dram_tensor("cc", [128, T], F32, kind="ExternalInput").ap()
    ss = nc.dram_tensor("ss", [128, T], F32, kind="ExternalInput").ap()
    sid = nc.dram_tensor("sid", [128, 128], F32R, kind="ExternalInput").ap()
    iden = nc.dram_tensor("iden", [128, 128], F32R, kind="ExternalInput").ap()
    tri = nc.dram_tensor("tri", [128, 128], F32, kind="ExternalInput").ap()
    out = nc.dram_tensor("out", [T, 1024], F32, kind="ExternalOutput").ap()

    with TileContext(nc) as tc:
        with tc.tile_pool(name="persist", bufs=1) as persist:
            qkrot = [[persist.tile([128, T], F32R, name=f"qkrot_{s}_{g}")
                      for g in range(2)] for s in range(4)]
            v_sb = persist.tile([128, NT, 256], F32R, name="v_sb")
            tri_sb = persist.tile([128, 128], F32, name="tri_sb")
            sid_sb = persist.tile([128, 128], F32R, name="sid_sb")
            nc.sync.dma_start(out=tri_sb[:], in_=tri)
            nc.sync.dma_start(out=sid_sb[:], in_=sid)

            # ---------------- phase P: projections + rope ----------------
            with tc.tile_pool(name="projw", bufs=1) as projw, \
                 tc.tile_pool(name="xcp", bufs=2) as xcp, \
                 tc.tile_pool(name="ropet", bufs=4) as ropet, \
                 tc.tile_pool(name="psq", bufs=3, space="PSUM") as psq, \
                 tc.tile_pool(name="psr", bufs=2, space="PSUM") as psr, \
                 tc.tile_pool(name="psv", bufs=2, space="PSUM") as psv:
                wqk_sb = projw.tile([128, 8, 1024], F32R, name="wqk_sb")
                wv_sb = projw.tile([128, 8, 256], F32R, name="wv_sb")
                cc_sb = projw.tile([128, T], F32, name="cc_sb")
                ss_sb = projw.tile([128, T], F32, name="ss_sb")
                for k in range(8):
                    nc.sync.dma_start(out=wqk_sb[:, k, :], in_=wqk[k])
                    nc.sync.dma_start(out=wv_sb[:, k, :], in_=wv[k])
                nc.sync.dma_start(out=cc_sb[:], in_=cc)
                nc.sync.dma_start(out=ss_sb[:], in_=ss)

                for j in range(NJ):
                    jsl = bass.ts(j, CH)
                    xc = xcp.tile([128, 8, CH], F32R, name="xc")
                    for k in range(8):
                        nc.sync.dma_start(out=xc[:, k, :], in_=xT[k, :, jsl])
                    for tt in range(4):
                        pv = psv.tile([128, 256], F32, name="pv")
                        for k in range(8):
                            nc.tensor.matmul(
                                pv[:], xc[:, k, bass.ts(tt, 128)], wv_sb[:, k, :],
                                start=(k == 0), stop=(k == 7))
                        nc.scalar.copy(out=v_sb[:, j * 4 + tt, :], in_=pv[:])
                    for s in range(4):
                        for g in range(2):
                            col = s * 256 + g * 128
                            pq = psq.tile([128, CH], F32, name="pq")
                            for k in range(8):
                                nc.tensor.matmul(
                                    pq[:], wqk_sb[:, k, col:col + 128], xc[:, k, :],
                                    start=(k == 0), stop=(k == 7))
                            wt = ropet.tile([128, CH], F32R, tag="wt", name="wt")
                            ut = ropet.tile([128, CH], F32R, tag="ut", name="ut")
                            nc.vector.tensor_mul(wt[:], pq[:], ss_sb[:, jsl])
                            nc.vector.tensor_mul(ut[:], pq[:], cc_sb[:, jsl])
                            pr = psr.tile([128, CH], F32, name="pr")
                            nc.tensor.matmul(pr[:], sid_sb[:], wt[:],
                                             start=True, stop=True)
                            nc.vector.scalar_tensor_tensor(
                                qkrot[s][g][:, jsl], pr[:], 1.0, ut[:],
                                mybir.AluOpType.mult, mybir.AluOpType.add)

            # ---------------- phase A + O ----------------
            with tc.tile_pool(name="patp", bufs=4) as patp, \
                 tc.tile_pool(name="s1cp", bufs=4) as s1cp, \
                 tc.tile_pool(name="outp", bufs=3) as outp, \
                 tc.tile_pool(name="attnp", bufs=1) as attnp, \
                 tc.tile_pool(name="pss", bufs=5, space="PSUM") as pss, \
                 tc.tile_pool(name="psy", bufs=2, space="PSUM") as psy, \
                 tc.tile_pool(name="pso", bufs=1, space="PSUM") as pso:
                yT = [attnp.tile([128, T], F32R, name=f"yT_{p}") for p in range(2)]
                wo_sb = attnp.tile([128, 2, 1024], F32R, name="wo_sb")
                nc.sync.dma_start(out=wo_sb[:], in_=wo.rearrange("p q n -> q p n"))
                for j in range(NJ):
                    jsl = bass.ts(j, CH)
                    for pair in range(2):
                        ytiles = [psy.tile([64, CH], F32, tag="y", name=f"yp{h}")
                                  for h in range(2)]
                        ni = 4 * j + 4
                        for i in range(ni):
                            diag = i >= 4 * j
                            c0 = 128 * (i - 4 * j) if diag else 0
                            isl = bass.ts(i, 128)
                            for h in range(2):
                                hrow = bass.ts(h, 64)
                                hrow2 = bass.ts(1 - h, 64)
                                s1 = pss.tile([128, CH], F32, tag="s", name="s1")
                                s2 = pss.tile([128, CH], F32, tag="s", name="s2")
                                nc.tensor.matmul(
                                    s1[:], qkrot[1][pair][hrow, isl],
                                    qkrot[0][pair][hrow, jsl],
                                    start=True, stop=True)
                                nc.tensor.matmul(
                                    s2[:], qkrot[3][pair][hrow2, isl],
                                    qkrot[2][pair][hrow2, jsl],
                                    start=True, stop=True)
                                s1c = s1cp.tile([128, CH], F32, name="s1c")
                                pat = patp.tile([128, CH], F32R, name="pat")
                                if not diag:
                                    nc.scalar.copy(out=s1c[:], in_=s1[:])
                                    nc.vector.tensor_mul(pat[:], s1c[:], s2[:])
                                else:
                                    nc.scalar.copy(out=s1c[:, c0:], in_=s1[:, c0:])
                                    nc.vector.tensor_mul(
                                        pat[:, c0:], s1c[:, c0:], s2[:, c0:])
                                    nc.vector.tensor_mul(
                                        pat[:, c0:c0 + 128],
                                        pat[:, c0:c0 + 128], tri_sb[:])
                                nc.tensor.matmul(
                                    ytiles[h][:, c0:],
                                    v_sb[:, i, bass.ts(2 * pair + h, 64)],
                                    pat[:, c0:],
                                    start=(i == 0), stop=(i == ni - 1),
                                    skip_group_check=True)
                        for h in range(2):
                            nc.scalar.copy(
                                out=yT[pair][bass.ts(h, 64), jsl],
                                in_=ytiles[h][:])
                    for tt in range(4 * j, 4 * j + 4):
                        tsl = bass.ts(tt, 128)
                        for co in range(2):
                            po = pso.tile([128, 512], F32, name="po")
                            nc.tensor.matmul(po[:], yT[0][:, tsl],
                                             wo_sb[:, 0, bass.ts(co, 512)],
                                             start=True, stop=False)
                            nc.tensor.matmul(po[:], yT[1][:, tsl],
                                             wo_sb[:, 1, bass.ts(co, 512)],
                                             start=False, stop=True)
                            ot = outp.tile([128, 512], F32, name="ot")
                            nc.scalar.copy(out=ot[:], in_=po[:])
                            nc.sync.dma_start(
                                out=out[tsl, bass.ts(co, 512)], in_=ot[:])
    return nc


# ------------------------------------------------------------- host side ---
def _rope_tables(T):
    inv_freq = (1.0 / (ROPE_BASE ** (np.arange(0, HEAD_DIM, 2, dtype=np.float32)
                                     / np.float32(HEAD_DIM)))).astype(np.float32)
    t = np.arange(T, dtype=np.float32)
    freqs = (t[:, None] * inv_freq[None, :]).astype(np.float32)
    cos = np.cos(freqs).astype(ml_dtypes.bfloat16).astype(np.float32)
    sin = np.sin(freqs).astype(ml_dtypes.bfloat16).astype(np.float32)
    cosT, sinT = cos.T, sin.T
    cc = np.ascontiguousarray(np.concatenate([cosT, cosT, cosT, cosT], axis=0))
    ss = np.ascontiguousarray(np.concatenate([-sinT, sinT, -sinT, sinT], axis=0))
    return cc, ss


def _const_tables():
    sid = np.zeros((128, 128), dtype=np.float32)
    for blk in range(2):
        for m in range(32):
            sid[blk * 64 + m + 32, blk * 64 + m] = 1.0
            sid[blk * 64 + m, blk * 64 + m + 32] = 1.0
    iden = np.eye(128, dtype=np.float32)
    r = np.arange(128)
    tri = (r[None, :] >= r[:, None]).astype(np.float32)
    return sid, iden, tri


def _make_in_maps(x, Wq1, Wk1, Wq2, Wk2, Wv, Wout, T):
    cc, ss = _rope_tables(T)
    sid, iden, tri = _const_tables()
    in_maps = []
    for core in range(8):
        b = core // 4
        hs = (core % 4) * 4
        xTb = np.ascontiguousarray(x[b].T).reshape(8, 128, T)
        cols = []
        for s, W in enumerate((Wq1, Wk1, Wq2, Wk2)):
            for g in range(2):
                hA, hB = hs + 2 * g, hs + 2 * g + 1
                if s >= 2:
                    hA, hB = hB, hA
                cols.append(W[hA * 64:(hA + 1) * 64, :].T)
                cols.append(W[hB * 64:(hB + 1) * 64, :].T)
        wqk = np.ascontiguousarray(
            np.concatenate(cols, axis=1)).reshape(8, 128, 1024)
        wv = np.ascontiguousarray(
            Wv[hs * 64:(hs + 4) * 64, :].T * np.float32(2.0 ** -12)
        ).reshape(8, 128, 256)
        wo = np.ascontiguousarray(
            Wout[:, hs * 64:(hs + 4) * 64].T).reshape(2, 128, 1024)
        in_maps.append({
            "xT": xTb.astype(np.float32), "wqk": wqk.astype(np.float32),
            "wv": wv.astype(np.float32), "wo": wo.astype(np.float32),
            "cc": cc, "ss": ss, "sid": sid, "iden": iden, "tri": tri,
        })
    return in_maps


_CACHED_NC = None


def kernel(x, Wq1, Wk1, Wq2, Wk2, Wv, Wout):
    global _CACHED_NC
    x = np.asarray(x, dtype=np.float32)
    args = [np.asarray(a, dtype=np.float32) for a in
            (Wq1, Wk1, Wq2, Wk2, Wv, Wout)]
    T = x.shape[1]
    if _CACHED_NC is None:
        nc = _build_program(T)
        _legalize_waits(nc, max_waits=1)
        _CACHED_NC = nc
    in_maps = _make_in_maps(x, *args, T)
    res = None
    last_err = None
    for attempt in range(3):
        try:
            res = run_bass_kernel_spmd(_CACHED_NC, in_maps, list(range(8)))
            break
        except Exception as e:  # transient NRT exec-unit wedge: retry
            last_err = e
            time.sleep(2.0)
    if res is None:
        raise last_err
    out = np.zeros((2, T, 1024), dtype=np.float32)
    for core in range(8):
        out[core // 4] += res.results[core]["out"]
    return out


# revision 6
# speedup vs baseline: 1.3789x; 1.0334x over previous
"""BilinearAttention (B=2, T=2048, C=1024, H=16, D=64) on 8 TRN2 NeuronCores.

Sharding: batch*heads across the 8 cores — core c handles batch b = c//4 and
heads [4*(c%4), 4*(c%4)+4).  Each core computes its heads' attention and a
partial output projection; the host sums the four per-batch partials.

Per-core kernel (Bass/Tile, all matmuls float32r = full PE rate, tf32-ish):
  phase P: xT chunk-streamed; q1/k1/q2/k2 projections in [o, t] layout; RoPE
           applied as q*cos + swap(q*sin') where the half-swap runs on the PE
           (permutation-matmul accumulated with an identity-matmul);
           v projected in [t, d] layout.  1/D^2 pattern scale is folded into
           Wv (exact power-of-two scaling).
  phase A: per head, scores computed transposed (patT[tk, tq]); the s1/s2
           matmuls of one head are row-packed on the PE (two concurrent K=64
           matmuls at row groups 0/64 — q2/k2 weights are head-swapped on the
           host to make the row groups line up).  ScalarE copies s1 from PSUM,
           VectorE forms s1*s2 with the causal mask (precomputed 128x128
           triangle applied only on diagonal tiles); the V-matmul accumulates
           yT[d, tq] in PSUM over tk tiles (strictly-upper tiles skipped).
  phase O: out[t, :] = yT.T @ WoutT slice, interleaved per chunk with phase A.
"""
import time
import numpy as np
import ml_dtypes

import concourse.bass as bass
import concourse.mybir as mybir
from concourse.tile import TileContext
from concourse.bass_utils import run_bass_kernel_spmd

F32 = mybir.dt.float32
F32R = mybir.dt.float32r

N_HEAD = 16
N_EMBD = 1024
HEAD_DIM = 64
ROPE_BASE = 10000.0
T_SEQ = 2048


def _legalize_waits(nc, max_waits=1):
    """walrus (CoreV3) accepts at most one sync-wait per instruction; hoist
    extras onto same-engine NoOps immediately before the instruction."""
    n_split = 0
    for f in nc.m.functions:
        for bb in f.blocks:
            insts = bb.instructions
            new_insts = []
            changed = False
            for inst in insts:
                si = inst.sync_info
                waits = list(si.on_wait) if si and si.on_wait else []
                if len(waits) > max_waits:
                    extra = waits[:-max_waits]
                    keep = waits[-max_waits:]
                    for i in range(0, len(extra), max_waits):
                        nop = mybir.InstNoOp(
                            name=f"{inst.name}_ws{i}",
                            engine=inst.engine,
                            sync_info=mybir.SyncInfo(
                                on_wait=extra[i:i + max_waits], on_update=[]
                            ),
                            bass_nofuse=True,
                        )
                        new_insts.append(nop)
                    inst.sync_info = mybir.SyncInfo(
                        on_wait=keep, on_update=list(si.on_update or [])
                    )
                    changed = True
                    n_split += 1
                new_insts.append(inst)
            if changed:
                bb.instructions = new_insts
    return n_split


def _build_program(T=T_SEQ):
    CH = 512
    NJ = T // CH
    NT = T // 128
    nc = bass.Bass("TRN2")

    xT = nc.dram_tensor("xT", [8, 128, T], F32R, kind="ExternalInput").ap()
    wqk = nc.dram_tensor("wqk", [8, 128, 1024], F32R, kind="ExternalInput").ap()
    wv = nc.dram_tensor("wv", [8, 128, 256], F32R, kind="ExternalInput").ap()
    wo = nc.dram_tensor("wo", [2, 128, 1024], F32R, kind="ExternalInput").ap()
    cc = nc.dram_tensor("cc", [128, T], F32, kind="ExternalInput").ap()
    ss = nc.dram_tensor("ss", [128, T], F32, kind="ExternalInput").ap()
    sid = nc.dram_tensor("sid", [128, 128], F32R, kind="ExternalInput").ap()
    iden = nc.dram_tensor("iden", [128, 128], F32R, kind="ExternalInput").ap()
    tri = nc.dram_tensor("tri", [128, 128], F32, kind="ExternalInput").ap()
    out = nc.dram_tensor("out", [T, 1024], F32, kind="ExternalOutput").ap()

    with TileContext(nc) as tc:
        with tc.tile_pool(name="persist", bufs=1) as persist:
            qkrot = [[persist.tile([128, T], F32R, name=f"qkrot_{s}_{g}")
                      for g in range(2)] for s in range(4)]
            v_sb = persist.tile([128, NT, 256], F32R, name="v_sb")
            tri_sb = persist.tile([128, 128], F32, name="tri_sb")
            sid_sb = persist.tile([128, 128], F32R, name="sid_sb")
            nc.sync.dma_start(out=tri_sb[:], in_=tri)
            nc.sync.dma_start(out=sid_sb[:], in_=sid)

            # ---------------- phase P: projections + rope ----------------
            with tc.tile_pool(name="projw", bufs=1) as projw, \
                 tc.tile_pool(name="xcp", bufs=2) as xcp, \
                 tc.tile_pool(name="ropet", bufs=4) as ropet, \
                 tc.tile_pool(name="psq", bufs=3, space="PSUM") as psq, \
                 tc.tile_pool(name="psr", bufs=2, space="PSUM") as psr, \
                 tc.tile_pool(name="psv", bufs=2, space="PSUM") as psv:
                wqk_sb = projw.tile([128, 8, 1024], F32R, name="wqk_sb")
                wv_sb = projw.tile([128, 8, 256], F32R, name="wv_sb")
                cc_sb = projw.tile([128, T], F32, name="cc_sb")
                ss_sb = projw.tile([128, T], F32, name="ss_sb")
                for k in range(8):
                    nc.sync.dma_start(out=wqk_sb[:, k, :], in_=wqk[k])
                    nc.sync.dma_start(out=wv_sb[:, k, :], in_=wv[k])
                nc.sync.dma_start(out=cc_sb[:], in_=cc)
                nc.sync.dma_start(out=ss_sb[:], in_=ss)

                for j in range(NJ):
                    jsl = bass.ts(j, CH)
                    xc = xcp.tile([128, 8, CH], F32R, name="xc")
                    for k in range(8):
                        nc.sync.dma_start(out=xc[:, k, :], in_=xT[k, :, jsl])
                    for tt in range(4):
                        pv = psv.tile([128, 256], F32, name="pv")
                        for k in range(8):
                            nc.tensor.matmul(
                                pv[:], xc[:, k, bass.ts(tt, 128)], wv_sb[:, k, :],
                                start=(k == 0), stop=(k == 7))
                        nc.scalar.copy(out=v_sb[:, j * 4 + tt, :], in_=pv[:])
                    for s in range(4):
                        for g in range(2):
                            col = s * 256 + g * 128
                            pq = psq.tile([128, CH], F32, name="pq")
                            for k in range(8):
                                nc.tensor.matmul(
                                    pq[:], wqk_sb[:, k, col:col + 128], xc[:, k, :],
                                    start=(k == 0), stop=(k == 7))
                            wt = ropet.tile([128, CH], F32R, tag="wt", name="wt")
                            ut = ropet.tile([128, CH], F32R, tag="ut", name="ut")
                            nc.vector.tensor_mul(wt[:], pq[:], ss_sb[:, jsl])
                            nc.vector.tensor_mul(ut[:], pq[:], cc_sb[:, jsl])
                            pr = psr.tile([128, CH], F32, name="pr")
                            nc.tensor.matmul(pr[:], sid_sb[:], wt[:],
                                             start=True, stop=True)
                            nc.vector.scalar_tensor_tensor(
                                qkrot[s][g][:, jsl], pr[:], 1.0, ut[:],
                                mybir.AluOpType.mult, mybir.AluOpType.add)

            # ---------------- phase A + O ----------------
            with tc.tile_pool(name="patp", bufs=4) as patp, \
                 tc.tile_pool(name="s1cp", bufs=4) as s1cp, \
                 tc.tile_pool(name="outp", bufs=3) as outp, \
                 tc.tile_pool(name="attnp", bufs=1) as attnp, \
                 tc.tile_pool(name="pss", bufs=5, space="PSUM") as pss, \
                 tc.tile_pool(name="psy", bufs=2, space="PSUM") as psy, \
                 tc.tile_pool(name="pso", bufs=1, space="PSUM") as pso:
                yT = [attnp.tile([128, T], F32R, name=f"yT_{p}") for p in range(2)]
                wo_sb = attnp.tile([128, 2, 1024], F32R, name="wo_sb")
                nc.sync.dma_start(out=wo_sb[:], in_=wo.rearrange("p q n -> q p n"))
                for j in range(NJ):
                    jsl = bass.ts(j, CH)
                    for pair in range(2):
                        ytiles = [psy.tile([64, CH], F32, tag="y", name=f"yp{h}")
                                  for h in range(2)]
                        ni = 4 * j + 4
                        for i in range(ni):
                            diag = i >= 4 * j
                            c0 = 128 * (i - 4 * j) if diag else 0
                            isl = bass.ts(i, 128)
                            for h in range(2):
                                hrow = bass.ts(h, 64)
                                hrow2 = bass.ts(1 - h, 64)
                                s1 = pss.tile([128, CH], F32, tag="s", name="s1")
                                s2 = pss.tile([128, CH], F32, tag="s", name="s2")
                                jq = bass.ds(j * CH + c0, CH - c0)
                                nc.tensor.matmul(
                                    s1[:, c0:], qkrot[1][pair][hrow, isl],
                                    qkrot[0][pair][hrow, jq],
                                    start=True, stop=True)
                                nc.tensor.matmul(
                                    s2[:, c0:], qkrot[3][pair][hrow2, isl],
                                    qkrot[2][pair][hrow2, jq],
                                    start=True, stop=True)
                                s1c = s1cp.tile([128, CH], F32, name="s1c")
                                pat = patp.tile([128, CH], F32R, name="pat")
                                if not diag:
                                    nc.scalar.copy(out=s1c[:], in_=s1[:])
                                    nc.vector.tensor_mul(pat[:], s1c[:], s2[:])
                                else:
                                    nc.scalar.copy(out=s1c[:, c0:], in_=s1[:, c0:])
                                    nc.vector.tensor_mul(
                                        pat[:, c0:], s1c[:, c0:], s2[:, c0:])
                                    nc.vector.tensor_mul(
                                        pat[:, c0:c0 + 128],
                                        pat[:, c0:c0 + 128], tri_sb[:])
                                nc.tensor.matmul(
                                    ytiles[h][:, c0:],
                                    v_sb[:, i, bass.ts(2 * pair + h, 64)],
                                    pat[:, c0:],
                                    start=(i == 0), stop=(i == ni - 1),
                                    skip_group_check=True)
                        for h in range(2):
                            nc.scalar.copy(
                                out=yT[pair][bass.ts(h, 64), jsl],
                                in_=ytiles[h][:])
                    for tt in range(4 * j, 4 * j + 4):
                        tsl = bass.ts(tt, 128)
                        for co in range(2):
                            po = pso.tile([128, 512], F32, name="po")
                            nc.tensor.matmul(po[:], yT[0][:, tsl],
                                             wo_sb[:, 0, bass.ts(co, 512)],
                                             start=True, stop=False)
                            nc.tensor.matmul(po[:], yT[1][:, tsl],
                                             wo_sb[:, 1, bass.ts(co, 512)],
                                             start=False, stop=True)
                            ot = outp.tile([128, 512], F32, name="ot")
                            nc.scalar.copy(out=ot[:], in_=po[:])
                            nc.sync.dma_start(
                                out=out[tsl, bass.ts(co, 512)], in_=ot[:])
    return nc


# ------------------------------------------------------------- host side ---
def _rope_tables(T):
    inv_freq = (1.0 / (ROPE_BASE ** (np.arange(0, HEAD_DIM, 2, dtype=np.float32)
                                     / np.float32(HEAD_DIM)))).astype(np.float32)
    t = np.arange(T, dtype=np.float32)
    freqs = (t[:, None] * inv_freq[None, :]).astype(np.float32)
    cos = np.cos(freqs).astype(ml_dtypes.bfloat16).astype(np.float32)
    sin = np.sin(freqs).astype(ml_dtypes.bfloat16).astype(np.float32)
    cosT, sinT = cos.T, sin.T
    cc = np.ascontiguousarray(np.concatenate([cosT, cosT, cosT, cosT], axis=0))
    ss = np.ascontiguousarray(np.concatenate([-sinT, sinT, -sinT, sinT], axis=0))
    return cc, ss


def _const_tables():
    sid = np.zeros((128, 128), dtype=np.float32)
    for blk in range(2):
        for m in range(32):
            sid[blk * 64 + m + 32, blk * 64 + m] = 1.0
            sid[blk * 64 + m, blk * 64 + m + 32] = 1.0
    iden = np.eye(128, dtype=np.float32)
    r = np.arange(128)
    tri = (r[None, :] >= r[:, None]).astype(np.float32)
    return sid, iden, tri


def _make_in_maps(x, Wq1, Wk1, Wq2, Wk2, Wv, Wout, T):
    cc, ss = _rope_tables(T)
    sid, iden, tri = _const_tables()
    in_maps = []
    for core in range(8):
        b = core // 4
        hs = (core % 4) * 4
        xTb = np.ascontiguousarray(x[b].T).reshape(8, 128, T)
        cols = []
        for s, W in enumerate((Wq1, Wk1, Wq2, Wk2)):
            for g in range(2):
                hA, hB = hs + 2 * g, hs + 2 * g + 1
                if s >= 2:
                    hA, hB = hB, hA
                cols.append(W[hA * 64:(hA + 1) * 64, :].T)
                cols.append(W[hB * 64:(hB + 1) * 64, :].T)
        wqk = np.ascontiguousarray(
            np.concatenate(cols, axis=1)).reshape(8, 128, 1024)
        wv = np.ascontiguousarray(
            Wv[hs * 64:(hs + 4) * 64, :].T * np.float32(2.0 ** -12)
        ).reshape(8, 128, 256)
        wo = np.ascontiguousarray(
            Wout[:, hs * 64:(hs + 4) * 64].T).reshape(2, 128, 1024)
        in_maps.append({
            "xT": xTb.astype(np.float32), "wqk": wqk.astype(np.float32),
            "wv": wv.astype(np.float32), "wo": wo.astype(np.float32),
            "cc": cc, "ss": ss, "sid": sid, "iden": iden, "tri": tri,
        })
    return in_maps


_CACHED_NC = None


def kernel(x, Wq1, Wk1, Wq2, Wk2, Wv, Wout):
    global _CACHED_NC
    x = np.asarray(x, dtype=np.float32)
    args = [np.asarray(a, dtype=np.float32) for a in
            (Wq1, Wk1, Wq2, Wk2, Wv, Wout)]
    T = x.shape[1]
    if _CACHED_NC is None:
        nc = _build_program(T)
        _legalize_waits(nc, max_waits=1)
        _CACHED_NC = nc
    in_maps = _make_in_maps(x, *args, T)
    res = None
    last_err = None
    for attempt in range(3):
        try:
            res = run_bass_kernel_spmd(_CACHED_NC, in_maps, list(range(8)))
            break
        except Exception as e:  # transient NRT exec-unit wedge: retry
            last_err = e
            time.sleep(2.0)
    if res is None:
        raise last_err
    out = np.zeros((2, T, 1024), dtype=np.float32)
    for core in range(8):
        out[core // 4] += res.results[core]["out"]
    return out
